# revision 5
# baseline (speedup 1.0000x reference)
"""Trainium2 Bass kernel for nn_ByteSequenceEmbedder.

Data-parallel across 8 NeuronCores: 2 sequences per core, weights replicated.

Per-core dataflow (all activations channels-on-partitions, "layout A"):
  embed   : ap_gather (GPSIMD) rows of tok_emb.T  -> x0 [128, T] (+ bpe gather, fused add)
  conv0   : 3 shifted matmuls per (T-chunk, co-chunk) accumulating in PSUM, ReLU+bias on ACT
  highway : 2 blocks x 2 layers; 8x4 matmuls per T-chunk, ReLU/Sigmoid evac, DVE combine
  conv1   : 12 matmuls per (T-chunk, co-chunk) + residual add
  pool    : ragged word max-pool via 3 ap_gathers (host-computed indices) + 2 DVE max
  proj    : 4x4x2 matmuls + bias -> out [512, 1024] per sequence (host transposes back)

Matmul operands are bf16 (f32 PSUM accumulation); final pooling source kept f32.
"""
import numpy as np

import concourse.bacc as bacc
import concourse.tile as tile
import concourse.mybir as mybir

BSZ, NW, T = 16, 1024, 3072
BED, WED = 128, 512
NH = 2
VOCAB = 264
BPE_MASK_IDX = 4
N_CORES = 8
SEQ_PER_CORE = BSZ // N_CORES
TP = T + 2          # padded with one zero halo column on each side
NCH = T // 512      # T-chunks of 512
BF16 = mybir.dt.bfloat16
F32 = mybir.dt.float32
I16 = mybir.dt.int16

_BF16_NP = mybir.dt.np(BF16)

_CACHE = {}


def _wrap_idx(idx):
    """ap_gather index layout: [128, n/16] int16, wrapped per 16-partition
    group and replicated across the 8 groups."""
    n = idx.shape[0]
    assert n % 16 == 0
    arr = idx.astype(np.int16).reshape(n // 16, 16).T  # [16, n/16]
    return np.tile(arr, (8, 1))                        # [128, n/16]


def _build_program():
    nc = bacc.Bacc("TRN2", target_bir_lowering=False, debug=False)

    din = {}
    def dram_in(name, shape, dt):
        din[name] = nc.dram_tensor(name, shape, dt, kind="ExternalInput").ap()
        return din[name]

    emb_t = dram_in("emb_t", [BED, VOCAB], F32)          # tok_emb.T
    w_c0 = dram_in("w_c0", [128, 3 * WED], BF16)         # [ci, k*512+co]
    w_c1 = dram_in("w_c1", [128, 4 * 3 * WED], BF16)     # [ci%128, (q*3+k)*512+co]
    w_hw = dram_in("w_hw", [128, 4 * 4 * 1024], BF16)    # [(bl*4+q)*1024 + co_out]
    w_pr = dram_in("w_pr", [128, 4 * WED], BF16)         # [q*512+co]
    b_c0 = dram_in("b_c0", [128, 4], F32)
    b_c1 = dram_in("b_c1", [128, 4], F32)
    b_hw = dram_in("b_hw", [128, 4 * 8], F32)            # [bl*8 + m]
    b_pr = dram_in("b_pr", [128, 4], F32)
    emb_idx = dram_in("emb_idx", [SEQ_PER_CORE, 128, T // 16], I16)
    bpe_idx = dram_in("bpe_idx", [SEQ_PER_CORE, 128, T // 16], I16)
    pool_idx = dram_in("pool_idx", [SEQ_PER_CORE, 128, 3 * (NW // 16)], I16)

    out = nc.dram_tensor("out", [SEQ_PER_CORE, WED, NW], F32, kind="ExternalOutput").ap()

    RELU = mybir.ActivationFunctionType.Relu
    SIGM = mybir.ActivationFunctionType.Sigmoid
    IDEN = mybir.ActivationFunctionType.Identity
    MAX = mybir.AluOpType.max
    ADD = mybir.AluOpType.add
    SUB = mybir.AluOpType.subtract
    MUL = mybir.AluOpType.mult

    with tile.TileContext(nc) as tc:
        with tc.tile_pool(name="wp", bufs=1) as wp, \
             tc.tile_pool(name="ap", bufs=1) as apool, \
             tc.tile_pool(name="tp", bufs=3) as tp, \
             tc.tile_pool(name="pp", bufs=8, space="PSUM") as pp:

            # ---- load weights/biases once ----
            t_emb = wp.tile([BED, VOCAB], F32)
            t_wc0 = wp.tile([128, 3 * WED], BF16)
            t_wc1 = wp.tile([128, 4 * 3 * WED], BF16)
            t_whw = wp.tile([128, 4 * 4 * 1024], BF16)
            t_wpr = wp.tile([128, 4 * WED], BF16)
            t_bc0 = wp.tile([128, 4], F32)
            t_bc1 = wp.tile([128, 4], F32)
            t_bhw = wp.tile([128, 4 * 8], F32)
            t_bpr = wp.tile([128, 4], F32)
            for t, d in ((t_emb, emb_t), (t_wc0, w_c0), (t_wc1, w_c1),
                         (t_whw, w_hw), (t_wpr, w_pr), (t_bc0, b_c0),
                         (t_bc1, b_c1), (t_bhw, b_hw), (t_bpr, b_pr)):
                nc.sync.dma_start(out=t[:], in_=d[:])

            def conv_block(X, Y, wt, bt, n_ci, halo_done):
                """Y[:, chunk m cols 1..T] = relu(conv(X) + b). X/Y: [128, n_ci*TP] views."""
                for n in range(NCH):
                    for m in range(4):
                        ps = pp.tile([128, 512], F32, tag="ps", name="ps")
                        nmm = n_ci * 3
                        i = 0
                        for q in range(n_ci):
                            for k in range(3):
                                if n_ci == 1:
                                    lhs = wt[:, k * WED + m * 128:k * WED + (m + 1) * 128]
                                else:
                                    lhs = wt[:, (q * 3 + k) * WED + m * 128:(q * 3 + k) * WED + (m + 1) * 128]
                                nc.tensor.matmul(
                                    out=ps[:], lhsT=lhs,
                                    rhs=X[:, q * TP + n * 512 + k:q * TP + n * 512 + k + 512],
                                    start=(i == 0), stop=(i == nmm - 1))
                                i += 1
                        dst = Y[:, m * TP + 1 + n * 512:m * TP + 1 + (n + 1) * 512]
                        nc.scalar.activation(out=dst, in_=ps[:], func=RELU,
                                             bias=bt[:, m:m + 1], scale=1.0)

            def highway_layer(X, Y, bl, y_dt_f32=False):
                """Y = g*relu(h) + (1-g)*X over [128, 4*TP] bf16 (Y maybe f32 chunks)."""
                for n in range(NCH):
                    pss = []
                    for m in range(8):
                        ps = pp.tile([128, 512], F32, tag="ps", name="ps")
                        for q in range(4):
                            base = (bl * 4 + q) * 1024 + m * 128
                            nc.tensor.matmul(
                                out=ps[:], lhsT=t_whw[:, base:base + 128],
                                rhs=X[:, q * TP + 1 + n * 512:q * TP + 1 + (n + 1) * 512],
                                start=(q == 0), stop=(q == 3))
                        pss.append(ps)
                    for c in range(4):
                        xs = X[:, c * TP + 1 + n * 512:c * TP + 1 + (n + 1) * 512]
                        h_t = tp.tile([128, 512], BF16, tag="h", name="h_t")
                        g_t = tp.tile([128, 512], BF16, tag="g", name="g_t")
                        d_t = tp.tile([128, 512], BF16, tag="d", name="d_t")
                        nc.scalar.activation(out=h_t[:], in_=pss[c][:], func=RELU,
                                             bias=t_bhw[:, bl * 8 + c:bl * 8 + c + 1], scale=1.0)
                        nc.scalar.activation(out=g_t[:], in_=pss[4 + c][:], func=SIGM,
                                             bias=t_bhw[:, bl * 8 + 4 + c:bl * 8 + 4 + c + 1], scale=1.0)
                        nc.vector.tensor_tensor(out=d_t[:], in0=h_t[:], in1=xs, op=SUB)
                        nc.vector.tensor_tensor(out=d_t[:], in0=d_t[:], in1=g_t[:], op=MUL)
                        ys = Y[:, c * TP + 1 + n * 512:c * TP + 1 + (n + 1) * 512]
                        nc.vector.tensor_tensor(out=ys, in0=d_t[:], in1=xs, op=ADD)

            import contextlib
            scope = nc.named_scope

            for s in range(SEQ_PER_CORE):
                # ---------- embedding ----------
                t_ei = apool.tile([128, T // 16], I16, tag="ei", name="t_ei")
                t_bi = apool.tile([128, T // 16], I16, tag="bi", name="t_bi")
                t_pi = apool.tile([128, 3 * (NW // 16)], I16, tag="pi", name="t_pi")
                nc.sync.dma_start(out=t_ei[:], in_=emb_idx[s])
                nc.sync.dma_start(out=t_bi[:], in_=bpe_idx[s])
                nc.sync.dma_start(out=t_pi[:], in_=pool_idx[s])

                # g1/g2 reuse the big activation-buffer slots (dead before those
                # buffers are first written this sequence)
                ctx_embed = scope(f"s{s}_embed"); ctx_embed.__enter__()
                g1 = apool.tile([128, TP], F32, tag="actB", name="g1")
                g2 = apool.tile([128, TP], F32, tag="actC", name="g2")
                nc.gpsimd.ap_gather(out_ap=g1[:, 1:T + 1], in_ap=t_emb[:],
                                    idxs_ap=t_ei[:], channels=128, num_elems=VOCAB,
                                    d=1, num_idxs=T)
                nc.gpsimd.ap_gather(out_ap=g2[:, 1:T + 1], in_ap=t_emb[:],
                                    idxs_ap=t_bi[:], channels=128, num_elems=VOCAB,
                                    d=1, num_idxs=T)
                x0 = apool.tile([128, TP], BF16, tag="x0", name="x0")
                nc.vector.memset(x0[:, 0:1], 0)
                nc.vector.memset(x0[:, TP - 1:TP], 0)
                nc.vector.tensor_tensor(out=x0[:, 1:T + 1], in0=g1[:, 1:T + 1],
                                        in1=g2[:, 1:T + 1], op=ADD)

                def act_buf(tag):
                    b = apool.tile([128, 4 * TP], BF16, tag=tag, name=tag)
                    for q in range(4):
                        nc.vector.memset(b[:, q * TP:q * TP + 1], 0)
                        nc.vector.memset(b[:, q * TP + TP - 1:q * TP + TP], 0)
                    return b

                ctx_embed.__exit__(None, None, None)
                # ---------- conv0 + highway block 0 ----------
                with scope(f"s{s}_conv0"):
                    x1 = act_buf("actA")
                    conv_block(x0, x1, t_wc0, t_bc0, 1, None)
                with scope(f"s{s}_hw0l0"):
                    x1b = act_buf("actB")
                    highway_layer(x1, x1b, 0)
                with scope(f"s{s}_hw0l1"):
                    x1c = act_buf("actC")
                    highway_layer(x1b, x1c, 1)

                # ---------- conv1 (+res) + highway block 1 ----------
                ctx_c1 = scope(f"s{s}_conv1"); ctx_c1.__enter__()
                x2p = act_buf("actA")
                for n in range(NCH):
                    for m in range(4):
                        ps = pp.tile([128, 512], F32, tag="ps", name="ps")
                        i = 0
                        for q in range(4):
                            for k in range(3):
                                lhs = t_wc1[:, (q * 3 + k) * WED + m * 128:(q * 3 + k) * WED + (m + 1) * 128]
                                nc.tensor.matmul(
                                    out=ps[:], lhsT=lhs,
                                    rhs=x1c[:, q * TP + n * 512 + k:q * TP + n * 512 + k + 512],
                                    start=(i == 0), stop=(i == 11))
                                i += 1
                        r_t = tp.tile([128, 512], BF16, tag="h", name="r_t")
                        nc.scalar.activation(out=r_t[:], in_=ps[:], func=RELU,
                                             bias=t_bc1[:, m:m + 1], scale=1.0)
                        xs = x1c[:, m * TP + 1 + n * 512:m * TP + 1 + (n + 1) * 512]
                        nc.vector.tensor_tensor(
                            out=x2p[:, m * TP + 1 + n * 512:m * TP + 1 + (n + 1) * 512],
                            in0=r_t[:], in1=xs, op=ADD)

                ctx_c1.__exit__(None, None, None)
                with scope(f"s{s}_hw1l0"):
                    x2b = act_buf("actB")
                    highway_layer(x2p, x2b, 2)
                ctx_hp = scope(f"s{s}_hw1l1pool"); ctx_hp.__enter__()

                # hw1 layer 1: emit per-chunk f32 output, then pool it immediately
                pooled = apool.tile([128, 4 * NW], BF16, tag="pooled", name="pooled")
                for c in range(4):
                    # reuses actA slot (x2p is dead once x2b exists)
                    x2f = apool.tile([128, TP], F32, tag="actA", name="x2f")
                    nc.vector.memset(x2f[:, 0:1], 0)
                    nc.vector.memset(x2f[:, TP - 1:TP], 0)
                    for n in range(NCH):
                        pss = []
                        for mm in (c, 4 + c):
                            ps = pp.tile([128, 512], F32, tag="ps", name="ps")
                            for q in range(4):
                                base = (3 * 4 + q) * 1024 + mm * 128
                                nc.tensor.matmul(
                                    out=ps[:], lhsT=t_whw[:, base:base + 128],
                                    rhs=x2b[:, q * TP + 1 + n * 512:q * TP + 1 + (n + 1) * 512],
                                    start=(q == 0), stop=(q == 3))
                            pss.append(ps)
                        xs = x2b[:, c * TP + 1 + n * 512:c * TP + 1 + (n + 1) * 512]
                        h_t = tp.tile([128, 512], BF16, tag="h", name="h_t")
                        g_t = tp.tile([128, 512], BF16, tag="g", name="g_t")
                        d_t = tp.tile([128, 512], BF16, tag="d", name="d_t")
                        nc.scalar.activation(out=h_t[:], in_=pss[0][:], func=RELU,
                                             bias=t_bhw[:, 3 * 8 + c:3 * 8 + c + 1], scale=1.0)
                        nc.scalar.activation(out=g_t[:], in_=pss[1][:], func=SIGM,
                                             bias=t_bhw[:, 3 * 8 + 4 + c:3 * 8 + 4 + c + 1], scale=1.0)
                        nc.vector.tensor_tensor(out=d_t[:], in0=h_t[:], in1=xs, op=SUB)
                        nc.vector.tensor_tensor(out=d_t[:], in0=d_t[:], in1=g_t[:], op=MUL)
                        nc.vector.tensor_tensor(
                            out=x2f[:, 1 + n * 512:1 + (n + 1) * 512],
                            in0=d_t[:], in1=xs, op=ADD)

                    # ---------- ragged max pool for this channel chunk ----------
                    ga = apool.tile([128, NW], F32, tag="ga", name="ga")
                    gb = apool.tile([128, NW], F32, tag="gb", name="gb")
                    gc_ = apool.tile([128, NW], F32, tag="gc", name="gc_")
                    for g_t_, off in ((ga, 0), (gb, 1), (gc_, 2)):
                        nc.gpsimd.ap_gather(
                            out_ap=g_t_[:], in_ap=x2f[:],
                            idxs_ap=t_pi[:, off * (NW // 16):(off + 1) * (NW // 16)],
                            channels=128, num_elems=TP, d=1, num_idxs=NW)
                    mx = apool.tile([128, NW], F32, tag="mx", name="mx")
                    nc.vector.tensor_tensor(out=mx[:], in0=ga[:], in1=gb[:], op=MAX)
                    nc.vector.tensor_tensor(out=pooled[:, c * NW:(c + 1) * NW],
                                            in0=mx[:], in1=gc_[:], op=MAX)

                ctx_hp.__exit__(None, None, None)
                # ---------- projection ----------
                for m in range(4):
                    o_t = apool.tile([128, NW], F32, tag="o", name="o_t")
                    for half in range(2):
                        ps = pp.tile([128, 512], F32, tag="ps", name="ps")
                        for q in range(4):
                            nc.tensor.matmul(
                                out=ps[:], lhsT=t_wpr[:, q * WED + m * 128:q * WED + (m + 1) * 128],
                                rhs=pooled[:, q * NW + half * 512:q * NW + (half + 1) * 512],
                                start=(q == 0), stop=(q == 3))
                        nc.scalar.activation(out=o_t[:, half * 512:(half + 1) * 512],
                                             in_=ps[:], func=IDEN,
                                             bias=t_bpr[:, m:m + 1], scale=1.0)
                    nc.sync.dma_start(out=out[s, m * 128:(m + 1) * 128, :], in_=o_t[:])

    nc.compile()
    return nc


def _prep_inputs(inputs):
    """Host-side: shard + convert to the kernel's DRAM tensor layouts."""
    byte_tokens = np.asarray(inputs["byte_tokens"], np.int64)
    bpe_mask = np.asarray(inputs["bpe_mask"], bool)
    pool_lengths = np.asarray(inputs["pool_lengths"], np.int64)
    tok_emb = np.asarray(inputs["tok_emb"], np.float32)

    def bf(x):
        return np.ascontiguousarray(np.asarray(x, np.float32).astype(_BF16_NP))

    conv0_W = np.asarray(inputs["conv0_W"], np.float32)   # [3,128,512]
    conv1_W = np.asarray(inputs["conv1_W"], np.float32)   # [3,512,512]
    hw0_W = np.asarray(inputs["hw0_W"], np.float32)       # [2,1024,512]
    hw1_W = np.asarray(inputs["hw1_W"], np.float32)
    proj_W = np.asarray(inputs["proj_W"], np.float32)     # [512,512]

    # w_c0: [ci=128, k*512+co]
    w_c0 = bf(conv0_W.transpose(1, 0, 2).reshape(128, 3 * WED))
    # w_c1: [ci%128, (q*3+k)*512+co]
    w_c1 = bf(conv1_W.transpose(1, 0, 2).reshape(4, 128, 3, WED)
              .transpose(1, 0, 2, 3).reshape(128, 4 * 3 * WED))
    # w_hw: Wt[ci, co_out] per (bl, q): [128, (bl*4+q)*1024+co]
    whw = np.empty((128, 16, 1024), np.float32)
    for bl, (blk, lay) in enumerate(((hw0_W, 0), (hw0_W, 1), (hw1_W, 0), (hw1_W, 1))):
        wt = blk[lay].T  # [512, 1024]
        for q in range(4):
            whw[:, bl * 4 + q, :] = wt[q * 128:(q + 1) * 128]
    w_hw = bf(whw.reshape(128, 16 * 1024))
    # w_pr: proj_W.T chunks
    w_pr = bf(proj_W.T.reshape(4, 128, WED).transpose(1, 0, 2).reshape(128, 4 * WED))

    def colchunks(b):  # [512] -> [128, 4]
        return np.ascontiguousarray(np.asarray(b, np.float32).reshape(4, 128).T)

    b_c0 = colchunks(inputs["conv0_b"])
    b_c1 = colchunks(inputs["conv1_b"])
    bhw = np.empty((128, 4, 8), np.float32)
    for bl, (blk, lay) in enumerate((("hw0_b", 0), ("hw0_b", 1), ("hw1_b", 0), ("hw1_b", 1))):
        b = np.asarray(inputs[blk], np.float32)[lay]  # [2048] -> wait [2*WED]=1024
        bhw[:, bl, 0:4] = b[:512].reshape(4, 128).T
        bhw[:, bl, 4:8] = b[512:1024].reshape(4, 128).T
    b_hw = np.ascontiguousarray(bhw.reshape(128, 32))
    b_pr = colchunks(inputs["proj_b"])

    emb_t = np.zeros((BED, VOCAB), np.float32)
    emb_t[:, :tok_emb.shape[0]] = np.asarray(tok_emb, np.float32).T

    shared = dict(emb_t=emb_t, w_c0=w_c0, w_c1=w_c1, w_hw=w_hw, w_pr=w_pr,
                  b_c0=b_c0, b_c1=b_c1, b_hw=b_hw, b_pr=b_pr)

    in_maps = []
    for core in range(N_CORES):
        m = dict(shared)
        eidx = np.empty((SEQ_PER_CORE, 128, T // 16), np.int16)
        bidx = np.empty((SEQ_PER_CORE, 128, T // 16), np.int16)
        pidx = np.empty((SEQ_PER_CORE, 128, 3 * (NW // 16)), np.int16)
        for s in range(SEQ_PER_CORE):
            b = core * SEQ_PER_CORE + s
            eidx[s] = _wrap_idx(byte_tokens[b])
            bidx[s] = _wrap_idx(np.where(bpe_mask[b], BPE_MASK_IDX, 0))
            pl = pool_lengths[b]
            cum = np.cumsum(pl)
            s_w = cum - pl
            nonempty = pl > 0
            ia = np.where(nonempty, s_w + 1, 0)
            ib = np.where(nonempty, s_w + 1 + (pl > 1), 0)
            ic = np.where(nonempty, s_w + pl, 0)
            pidx[s] = np.concatenate(
                [_wrap_idx(ia), _wrap_idx(ib), _wrap_idx(ic)], axis=1)
        m["emb_idx"] = eidx
        m["bpe_idx"] = bidx
        m["pool_idx"] = pidx
        in_maps.append(m)
    return in_maps


def kernel(**inputs) -> np.ndarray:
    from concourse.bass_utils import run_bass_kernel_spmd

    if "nc" not in _CACHE:
        _CACHE["nc"] = _build_program()
    nc = _CACHE["nc"]

    in_maps = _prep_inputs(inputs)
    res = run_bass_kernel_spmd(nc, in_maps, list(range(N_CORES)))
    full = np.empty((BSZ, NW, WED), np.float32)
    for core in range(N_CORES):
        o = np.asarray(res.results[core]["out"], np.float32)  # [2, 512, 1024]
        for s in range(SEQ_PER_CORE):
            full[core * SEQ_PER_CORE + s] = o[s].T
    return full


# revision 8
# speedup vs baseline: 2.4334x; 2.4334x over previous
"""Trainium2 Bass kernel for nn_ByteSequenceEmbedder.

Data-parallel across 8 NeuronCores: 2 sequences per core, weights replicated.

Per-core dataflow (all activations channels-on-partitions, "layout A" [C, T]):
  embed   : one-hot matmul — tokens broadcast [128,T] (host), DVE is_equal vs
            per-partition iota -> onehot chunks; PE: tok_emb-chunks.T @ onehot
            accumulated in PSUM (+ K=1 matmul adding bpe-marker row)
  conv0   : 3 shifted matmuls per (T-chunk, co-chunk) accumulating in PSUM,
            ReLU+bias fused into the ACT PSUM->SBUF evacuation
  highway : 2 blocks x 2 layers; 8x4 matmuls per T-chunk, ReLU/Sigmoid evac,
            DVE combine x' = g*(relu(h)-x)+x
  conv1   : 12 matmuls per (T-chunk, co-chunk) + residual add
  pool    : ragged word max-pool as masked shifted max:
            msel[t] = max(x2[t], x2[t+1]+A1[t], x2[t+2]+A2[t]) with host-built
            additive masks (0 where word@t has len>j, else -1e30)
  proj    : projection applied over ALL T positions; host selects column s_w
            per word while unsharding (empty pools -> proj_b row)

Matmul operands are bf16 (f32 PSUM accumulation).
"""
import numpy as np

import concourse.bacc as bacc
import concourse.tile as tile
import concourse.mybir as mybir

BSZ, NW, T = 16, 1024, 3072
BED, WED = 128, 512
VOCAB = 264
BPE_MASK_IDX = 4
N_CORES = 8
SEQ_PER_CORE = BSZ // N_CORES
TP = T + 2          # conv buffers: one zero halo col each side
TP2 = T + 4         # pooling source: 1 left + 3 right halo cols
NCH = T // 512      # T-chunks of 512
BF16 = mybir.dt.bfloat16
F16 = mybir.dt.float16
F32 = mybir.dt.float32

_BF16_NP = mybir.dt.np(BF16)
_F16_NP = np.float16
NEG_BIG = -1e30

_CACHE = {}


def _build_program():
    nc = bacc.Bacc("TRN2", target_bir_lowering=False, debug=False)

    def dram_in(name, shape, dt):
        return nc.dram_tensor(name, shape, dt, kind="ExternalInput").ap()

    emb_lhs = dram_in("emb_lhs", [128, 3 * 128], BF16)   # tok_emb row-chunks
    emb_row4 = dram_in("emb_row4", [1, 128], BF16)       # tok_emb[4]
    iota_c = dram_in("iota_c", [128, 3], F32)            # per-partition vocab iota
    w_c0 = dram_in("w_c0", [128, 3 * WED], BF16)         # [ci, k*512+co]
    w_c1 = dram_in("w_c1", [128, 4 * 3 * WED], BF16)     # [ci%128, (q*3+k)*512+co]
    w_hw = dram_in("w_hw", [128, 4 * 4 * 1024], BF16)    # [(bl*4+q)*1024 + co_out]
    w_pr = dram_in("w_pr", [128, 4 * WED], BF16)         # [q*512+co]
    b_c0 = dram_in("b_c0", [128, 4], F32)
    b_c1 = dram_in("b_c1", [128, 4], F32)
    b_hw = dram_in("b_hw", [128, 4 * 8], F32)            # [bl*8 + m]
    b_pr = dram_in("b_pr", [128, 4], F32)
    tok_bc = dram_in("tok_bc", [SEQ_PER_CORE, 128, T], F16)   # tokens bcast over partitions
    bpe_row = dram_in("bpe_row", [SEQ_PER_CORE, 1, T], BF16)  # bpe mask 0/1
    a_msk = dram_in("a_msk", [SEQ_PER_CORE, 128, 2 * T], BF16)  # pooling additive masks

    out = nc.dram_tensor("out", [SEQ_PER_CORE, WED, T], F32, kind="ExternalOutput").ap()

    RELU = mybir.ActivationFunctionType.Relu
    SIGM = mybir.ActivationFunctionType.Sigmoid
    IDEN = mybir.ActivationFunctionType.Identity
    MAX = mybir.AluOpType.max
    ADD = mybir.AluOpType.add
    SUB = mybir.AluOpType.subtract
    MUL = mybir.AluOpType.mult
    ISEQ = mybir.AluOpType.is_equal

    with tile.TileContext(nc) as tc:
        with tc.tile_pool(name="wp", bufs=1) as wp, \
             tc.tile_pool(name="ap", bufs=1) as apool, \
             tc.tile_pool(name="tp", bufs=3) as tp, \
             tc.tile_pool(name="pp", bufs=8, space="PSUM") as pp:

            # ---- load weights/biases once (sync queue; small/early first) ----
            t_embA = wp.tile([128, 3 * 128], BF16)
            t_row4 = wp.tile([1, 128], BF16)
            t_iota = wp.tile([128, 3], F32)
            t_bc0 = wp.tile([128, 4], F32)
            t_bc1 = wp.tile([128, 4], F32)
            t_bhw = wp.tile([128, 4 * 8], F32)
            t_bpr = wp.tile([128, 4], F32)
            t_wc0 = wp.tile([128, 3 * WED], BF16)
            t_wc1 = wp.tile([128, 4 * 3 * WED], BF16)
            t_whw = wp.tile([128, 4 * 4 * 1024], BF16)
            t_wpr = wp.tile([128, 4 * WED], BF16)
            for t, d in ((t_embA, emb_lhs), (t_row4, emb_row4), (t_iota, iota_c),
                         (t_bc0, b_c0), (t_bc1, b_c1), (t_bhw, b_hw), (t_bpr, b_pr),
                         (t_wc0, w_c0), (t_wc1, w_c1), (t_whw, w_hw), (t_wpr, w_pr)):
                nc.sync.dma_start(out=t[:], in_=d[:])

            def conv_block(X, Y, wt, bt, n_ci):
                """Y[:, chunk m cols 1..T] = relu(conv(X) + b)."""
                for n in range(NCH):
                    for m in range(4):
                        ps = pp.tile([128, 512], F32, tag="ps", name="ps")
                        nmm = n_ci * 3
                        i = 0
                        for q in range(n_ci):
                            for k in range(3):
                                lhs = wt[:, (q * 3 + k) * WED + m * 128:(q * 3 + k) * WED + (m + 1) * 128]
                                nc.tensor.matmul(
                                    out=ps[:], lhsT=lhs,
                                    rhs=X[:, q * TP + n * 512 + k:q * TP + n * 512 + k + 512],
                                    start=(i == 0), stop=(i == nmm - 1))
                                i += 1
                        dst = Y[:, m * TP + 1 + n * 512:m * TP + 1 + (n + 1) * 512]
                        nc.scalar.activation(out=dst, in_=ps[:], func=RELU,
                                             bias=bt[:, m:m + 1], scale=1.0)

            def highway_layer(X, Y, bl, ytp=TP):
                """Y = g*relu(h) + (1-g)*X; X [128, 4*TP], Y [128, 4*ytp]."""
                for n in range(NCH):
                    pss = []
                    for m in range(8):
                        ps = pp.tile([128, 512], F32, tag="ps", name="ps")
                        for q in range(4):
                            base = (bl * 4 + q) * 1024 + m * 128
                            nc.tensor.matmul(
                                out=ps[:], lhsT=t_whw[:, base:base + 128],
                                rhs=X[:, q * TP + 1 + n * 512:q * TP + 1 + (n + 1) * 512],
                                start=(q == 0), stop=(q == 3))
                        pss.append(ps)
                    for c in range(4):
                        xs = X[:, c * TP + 1 + n * 512:c * TP + 1 + (n + 1) * 512]
                        h_t = tp.tile([128, 512], BF16, tag="h", name="h_t")
                        g_t = tp.tile([128, 512], BF16, tag="g", name="g_t")
                        d_t = tp.tile([128, 512], BF16, tag="d", name="d_t")
                        nc.scalar.activation(out=h_t[:], in_=pss[c][:], func=RELU,
                                             bias=t_bhw[:, bl * 8 + c:bl * 8 + c + 1], scale=1.0)
                        nc.scalar.activation(out=g_t[:], in_=pss[4 + c][:], func=SIGM,
                                             bias=t_bhw[:, bl * 8 + 4 + c:bl * 8 + 4 + c + 1], scale=1.0)
                        nc.vector.tensor_tensor(out=d_t[:], in0=h_t[:], in1=xs, op=SUB)
                        nc.vector.tensor_tensor(out=d_t[:], in0=d_t[:], in1=g_t[:], op=MUL)
                        ys = Y[:, c * ytp + 1 + n * 512:c * ytp + 1 + (n + 1) * 512]
                        nc.vector.tensor_tensor(out=ys, in0=d_t[:], in1=xs, op=ADD)

            scope = nc.named_scope

            for s in range(SEQ_PER_CORE):
                # ---------- embedding (one-hot matmul) ----------
                ctx = scope(f"s{s}_embed"); ctx.__enter__()
                t_tok = apool.tile([128, T], F16, tag="tok", name="t_tok")
                t_bpe = apool.tile([1, T], BF16, tag="bpe", name="t_bpe")
                t_am = apool.tile([128, 2 * T], BF16, tag="am", name="t_am")
                nc.scalar.dma_start(out=t_tok[:], in_=tok_bc[s])
                nc.scalar.dma_start(out=t_bpe[:], in_=bpe_row[s])
                nc.scalar.dma_start(out=t_am[:], in_=a_msk[s])

                x0 = apool.tile([128, TP], BF16, tag="x0", name="x0")
                nc.vector.memset(x0[:, 0:1], 0)
                nc.vector.memset(x0[:, TP - 1:TP], 0)
                for n in range(NCH):
                    oh1 = tp.tile([128, 512], BF16, tag="oh1", name="oh1")
                    oh2 = tp.tile([128, 512], BF16, tag="oh2", name="oh2")
                    oh3 = tp.tile([8, 512], BF16, tag="oh3", name="oh3")
                    tb = t_tok[:, n * 512:(n + 1) * 512]
                    nc.vector.tensor_scalar(out=oh1[:], in0=tb, scalar1=t_iota[:, 0:1],
                                            scalar2=None, op0=ISEQ)
                    nc.vector.tensor_scalar(out=oh2[:], in0=tb, scalar1=t_iota[:, 1:2],
                                            scalar2=None, op0=ISEQ)
                    nc.vector.tensor_scalar(out=oh3[:], in0=t_tok[0:8, n * 512:(n + 1) * 512],
                                            scalar1=t_iota[0:8, 2:3], scalar2=None, op0=ISEQ)
                    ps = pp.tile([128, 512], F32, tag="ps", name="ps")
                    nc.tensor.matmul(out=ps[:], lhsT=t_embA[:, 0:128], rhs=oh1[:],
                                     start=True, stop=False)
                    nc.tensor.matmul(out=ps[:], lhsT=t_embA[:, 128:256], rhs=oh2[:],
                                     start=False, stop=False)
                    nc.tensor.matmul(out=ps[:], lhsT=t_embA[0:8, 256:384], rhs=oh3[:],
                                     start=False, stop=False)
                    nc.tensor.matmul(out=ps[:], lhsT=t_row4[:], rhs=t_bpe[:, n * 512:(n + 1) * 512],
                                     start=False, stop=True)
                    nc.scalar.activation(out=x0[:, 1 + n * 512:1 + (n + 1) * 512],
                                         in_=ps[:], func=IDEN, bias=0.0, scale=1.0)
                ctx.__exit__(None, None, None)

                def act_buf(tag, w=TP, extra_halo=0):
                    b = apool.tile([128, 4 * w], BF16, tag=tag, name=tag)
                    for q in range(4):
                        nc.vector.memset(b[:, q * w:q * w + 1], 0)
                        nc.vector.memset(b[:, q * w + 1 + T:(q + 1) * w], 0)
                    return b

                # ---------- conv0 + highway block 0 ----------
                with scope(f"s{s}_conv0"):
                    x1 = act_buf("actA")
                    conv_block(x0, x1, t_wc0, t_bc0, 1)
                with scope(f"s{s}_hw0l0"):
                    x1b = act_buf("actB")
                    highway_layer(x1, x1b, 0)
                with scope(f"s{s}_hw0l1"):
                    x1c = act_buf("actC")
                    highway_layer(x1b, x1c, 1)

                # ---------- conv1 (+res) + highway block 1 ----------
                ctx = scope(f"s{s}_conv1"); ctx.__enter__()
                x2p = act_buf("actA")
                for n in range(NCH):
                    for m in range(4):
                        ps = pp.tile([128, 512], F32, tag="ps", name="ps")
                        i = 0
                        for q in range(4):
                            for k in range(3):
                                lhs = t_wc1[:, (q * 3 + k) * WED + m * 128:(q * 3 + k) * WED + (m + 1) * 128]
                                nc.tensor.matmul(
                                    out=ps[:], lhsT=lhs,
                                    rhs=x1c[:, q * TP + n * 512 + k:q * TP + n * 512 + k + 512],
                                    start=(i == 0), stop=(i == 11))
                                i += 1
                        r_t = tp.tile([128, 512], BF16, tag="h", name="r_t")
                        nc.scalar.activation(out=r_t[:], in_=ps[:], func=RELU,
                                             bias=t_bc1[:, m:m + 1], scale=1.0)
                        xs = x1c[:, m * TP + 1 + n * 512:m * TP + 1 + (n + 1) * 512]
                        nc.vector.tensor_tensor(
                            out=x2p[:, m * TP + 1 + n * 512:m * TP + 1 + (n + 1) * 512],
                            in0=r_t[:], in1=xs, op=ADD)
                ctx.__exit__(None, None, None)

                with scope(f"s{s}_hw1l0"):
                    x2b = act_buf("actB")
                    highway_layer(x2p, x2b, 2)
                with scope(f"s{s}_hw1l1"):
                    x2 = act_buf("actC", w=TP2)
                    highway_layer(x2b, x2, 3, ytp=TP2)

                # ---------- ragged max pool (masked shifted max) ----------
                ctx = scope(f"s{s}_pool"); ctx.__enter__()
                msel = apool.tile([128, 4 * T], BF16, tag="actB", name="msel")
                for c in range(4):
                    for n in range(NCH):
                        lo, hi = n * 512, (n + 1) * 512
                        base = c * TP2 + 1
                        s1 = tp.tile([128, 512], BF16, tag="s1", name="s1")
                        s2 = tp.tile([128, 512], BF16, tag="s2", name="s2")
                        nc.vector.tensor_tensor(out=s1[:], in0=x2[:, base + 1 + lo:base + 1 + hi],
                                                in1=t_am[:, lo:hi], op=ADD)
                        nc.vector.tensor_tensor(out=s2[:], in0=x2[:, base + 2 + lo:base + 2 + hi],
                                                in1=t_am[:, T + lo:T + hi], op=ADD)
                        nc.vector.tensor_tensor(out=s1[:], in0=s1[:], in1=s2[:], op=MAX)
                        nc.vector.tensor_tensor(out=msel[:, c * T + lo:c * T + hi],
                                                in0=s1[:], in1=x2[:, base + lo:base + hi], op=MAX)
                ctx.__exit__(None, None, None)

                # ---------- projection over all T positions ----------
                ctx = scope(f"s{s}_proj"); ctx.__enter__()
                for m in range(4):
                    o_t = apool.tile([128, T], F32, tag="o", name="o_t", bufs=2)
                    for n in range(NCH):
                        ps = pp.tile([128, 512], F32, tag="ps", name="ps")
                        for q in range(4):
                            nc.tensor.matmul(
                                out=ps[:], lhsT=t_wpr[:, q * WED + m * 128:q * WED + (m + 1) * 128],
                                rhs=msel[:, q * T + n * 512:q * T + (n + 1) * 512],
                                start=(q == 0), stop=(q == 3))
                        nc.scalar.activation(out=o_t[:, n * 512:(n + 1) * 512],
                                             in_=ps[:], func=IDEN,
                                             bias=t_bpr[:, m:m + 1], scale=1.0)
                    nc.sync.dma_start(out=out[s, m * 128:(m + 1) * 128, :], in_=o_t[:])
                ctx.__exit__(None, None, None)

    nc.compile()
    return nc


def _prep_inputs(inputs):
    """Host-side: shard + convert to the kernel's DRAM tensor layouts."""
    byte_tokens = np.asarray(inputs["byte_tokens"], np.int64)
    bpe_mask = np.asarray(inputs["bpe_mask"], bool)
    pool_lengths = np.asarray(inputs["pool_lengths"], np.int64)
    tok_emb = np.asarray(inputs["tok_emb"], np.float32)

    def bf(x):
        return np.ascontiguousarray(np.asarray(x, np.float32).astype(_BF16_NP))

    conv0_W = np.asarray(inputs["conv0_W"], np.float32)   # [3,128,512]
    conv1_W = np.asarray(inputs["conv1_W"], np.float32)   # [3,512,512]
    hw0_W = np.asarray(inputs["hw0_W"], np.float32)       # [2,1024,512]
    hw1_W = np.asarray(inputs["hw1_W"], np.float32)
    proj_W = np.asarray(inputs["proj_W"], np.float32)     # [512,512]

    w_c0 = bf(conv0_W.transpose(1, 0, 2).reshape(128, 3 * WED))
    w_c1 = bf(conv1_W.transpose(1, 0, 2).reshape(4, 128, 3, WED)
              .transpose(1, 0, 2, 3).reshape(128, 4 * 3 * WED))
    whw = np.empty((128, 16, 1024), np.float32)
    for bl, (blk, lay) in enumerate(((hw0_W, 0), (hw0_W, 1), (hw1_W, 0), (hw1_W, 1))):
        wt = blk[lay].T  # [512, 1024]
        for q in range(4):
            whw[:, bl * 4 + q, :] = wt[q * 128:(q + 1) * 128]
    w_hw = bf(whw.reshape(128, 16 * 1024))
    w_pr = bf(proj_W.T.reshape(4, 128, WED).transpose(1, 0, 2).reshape(128, 4 * WED))

    def colchunks(b):  # [512] -> [128, 4]
        return np.ascontiguousarray(np.asarray(b, np.float32).reshape(4, 128).T)

    b_c0 = colchunks(inputs["conv0_b"])
    b_c1 = colchunks(inputs["conv1_b"])
    bhw = np.empty((128, 4, 8), np.float32)
    for bl, (blk, lay) in enumerate((("hw0_b", 0), ("hw0_b", 1), ("hw1_b", 0), ("hw1_b", 1))):
        b = np.asarray(inputs[blk], np.float32)[lay]      # [1024]
        bhw[:, bl, 0:4] = b[:512].reshape(4, 128).T
        bhw[:, bl, 4:8] = b[512:1024].reshape(4, 128).T
    b_hw = np.ascontiguousarray(bhw.reshape(128, 32))
    b_pr = colchunks(inputs["proj_b"])

    # embedding table as lhsT row-chunks [128, 3*128]
    emb_lhs = np.zeros((128, 3 * 128), np.float32)
    emb_lhs[:, 0:128] = tok_emb[0:128]
    emb_lhs[:, 128:256] = tok_emb[128:256]
    emb_lhs[0:8, 256:384] = tok_emb[256:264]
    emb_lhs = bf(emb_lhs)
    emb_row4 = bf(tok_emb[BPE_MASK_IDX:BPE_MASK_IDX + 1, :])  # [1, 128]
    iota_c = np.empty((128, 3), np.float32)
    p = np.arange(128)
    for j in range(3):
        iota_c[:, j] = (j * 128 + p).astype(np.float32)

    shared = dict(emb_lhs=emb_lhs, emb_row4=emb_row4, iota_c=iota_c,
                  w_c0=w_c0, w_c1=w_c1, w_hw=w_hw, w_pr=w_pr,
                  b_c0=b_c0, b_c1=b_c1, b_hw=b_hw, b_pr=b_pr)

    in_maps = []
    meta = []
    for core in range(N_CORES):
        m = dict(shared)
        tok = np.empty((SEQ_PER_CORE, 128, T), _F16_NP)
        bpe = np.empty((SEQ_PER_CORE, 1, T), _BF16_NP)
        amsk = np.empty((SEQ_PER_CORE, 128, 2 * T), _BF16_NP)
        for s in range(SEQ_PER_CORE):
            b = core * SEQ_PER_CORE + s
            tok[s] = np.broadcast_to(byte_tokens[b].astype(_F16_NP), (128, T))
            bpe[s, 0] = (bpe_mask[b]).astype(_BF16_NP)
            pl = pool_lengths[b]
            cum = np.cumsum(pl)
            s_w = (cum - pl)
            a1 = np.full(T, NEG_BIG, np.float32)
            a2 = np.full(T, NEG_BIG, np.float32)
            st = s_w[pl > 1]
            a1[st[st < T]] = 0.0
            st = s_w[pl > 2]
            a2[st[st < T]] = 0.0
            amsk[s, :, 0:T] = np.broadcast_to(a1.astype(_BF16_NP), (128, T))
            amsk[s, :, T:2 * T] = np.broadcast_to(a2.astype(_BF16_NP), (128, T))
            meta.append((s_w, pl))
        m["tok_bc"] = tok
        m["bpe_row"] = bpe
        m["a_msk"] = amsk
        in_maps.append(m)
    return in_maps, meta


def kernel(**inputs) -> np.ndarray:
    from concourse.bass_utils import run_bass_kernel_spmd

    if "nc" not in _CACHE:
        _CACHE["nc"] = _build_program()
    nc = _CACHE["nc"]

    in_maps, meta = _prep_inputs(inputs)
    res = run_bass_kernel_spmd(nc, in_maps, list(range(N_CORES)))

    proj_b = np.asarray(inputs["proj_b"], np.float32)
    full = np.empty((BSZ, NW, WED), np.float32)
    for core in range(N_CORES):
        o = np.asarray(res.results[core]["out"], np.float32)  # [2, 512, T]
        for s in range(SEQ_PER_CORE):
            b = core * SEQ_PER_CORE + s
            s_w, pl = meta[b]
            cols = np.clip(s_w, 0, T - 1)
            full[b] = o[s][:, cols].T
            if (pl == 0).any():
                full[b][pl == 0] = proj_b
    return full


# revision 9
# speedup vs baseline: 2.5869x; 1.0631x over previous
"""Trainium2 Bass kernel for nn_ByteSequenceEmbedder.

Data-parallel across 8 NeuronCores: 2 sequences per core, weights replicated.

Per-core dataflow (all activations channels-on-partitions, "layout A" [C, T]):
  embed   : one-hot matmul — tokens broadcast [128,T] (host), DVE is_equal vs
            per-partition iota -> onehot chunks; PE: tok_emb-chunks.T @ onehot
            accumulated in PSUM (+ K=1 matmul adding bpe-marker row)
  conv0   : 3 shifted matmuls per (T-chunk, co-chunk) accumulating in PSUM,
            ReLU+bias fused into the ACT PSUM->SBUF evacuation
  highway : 2 blocks x 2 layers; 8x4 matmuls per T-chunk, ReLU/Sigmoid evac,
            DVE combine x' = g*(relu(h)-x)+x
  conv1   : 12 matmuls per (T-chunk, co-chunk) + residual add
  pool    : ragged word max-pool as masked shifted max:
            msel[t] = max(x2[t], x2[t+1]+A1[t], x2[t+2]+A2[t]) with host-built
            additive masks (0 where word@t has len>j, else -1e30)
  proj    : projection applied over ALL T positions; host selects column s_w
            per word while unsharding (empty pools -> proj_b row)

Matmul operands are bf16 (f32 PSUM accumulation).
"""
import numpy as np

import concourse.bacc as bacc
import concourse.tile as tile
import concourse.mybir as mybir

BSZ, NW, T = 16, 1024, 3072
BED, WED = 128, 512
VOCAB = 264
BPE_MASK_IDX = 4
N_CORES = 8
SEQ_PER_CORE = BSZ // N_CORES
TP = T + 2          # conv buffers: one zero halo col each side
TP2 = T + 4         # pooling source: 1 left + 3 right halo cols
NCH = T // 512      # T-chunks of 512
BF16 = mybir.dt.bfloat16
F16 = mybir.dt.float16
F32 = mybir.dt.float32

_BF16_NP = mybir.dt.np(BF16)
_F16_NP = np.float16
NEG_BIG = -1e30

_CACHE = {}


def _build_program():
    nc = bacc.Bacc("TRN2", target_bir_lowering=False, debug=False)

    def dram_in(name, shape, dt):
        return nc.dram_tensor(name, shape, dt, kind="ExternalInput").ap()

    emb_lhs = dram_in("emb_lhs", [128, 3 * 128], BF16)   # tok_emb row-chunks
    emb_row4 = dram_in("emb_row4", [1, 128], BF16)       # tok_emb[4]
    iota_c = dram_in("iota_c", [128, 3], F32)            # per-partition vocab iota
    w_c0 = dram_in("w_c0", [128, 3 * WED], BF16)         # [ci, k*512+co]
    w_c1 = dram_in("w_c1", [128, 4 * 3 * WED], BF16)     # [ci%128, (q*3+k)*512+co]
    w_hw = dram_in("w_hw", [128, 4 * 4 * 1024], BF16)    # [(bl*4+q)*1024 + co_out]
    w_pr = dram_in("w_pr", [128, 4 * WED], BF16)         # [q*512+co]
    b_c0 = dram_in("b_c0", [128, 4], F32)
    b_c1 = dram_in("b_c1", [128, 4], F32)
    b_hw = dram_in("b_hw", [128, 4 * 8], F32)            # [bl*8 + m]
    b_pr = dram_in("b_pr", [128, 4], F32)
    tok_bc = dram_in("tok_bc", [SEQ_PER_CORE, 128, T], F16)   # tokens bcast over partitions
    bpe_row = dram_in("bpe_row", [SEQ_PER_CORE, 1, T], BF16)  # bpe mask 0/1
    a_msk = dram_in("a_msk", [SEQ_PER_CORE, 128, 2 * T], BF16)  # pooling additive masks

    out = nc.dram_tensor("out", [SEQ_PER_CORE, WED, T], F32, kind="ExternalOutput").ap()

    RELU = mybir.ActivationFunctionType.Relu
    SIGM = mybir.ActivationFunctionType.Sigmoid
    IDEN = mybir.ActivationFunctionType.Identity
    MAX = mybir.AluOpType.max
    ADD = mybir.AluOpType.add
    SUB = mybir.AluOpType.subtract
    MUL = mybir.AluOpType.mult
    ISEQ = mybir.AluOpType.is_equal

    with tile.TileContext(nc) as tc:
        with tc.tile_pool(name="wp", bufs=1) as wp, \
             tc.tile_pool(name="ap", bufs=1) as apool, \
             tc.tile_pool(name="tp", bufs=3) as tp, \
             tc.tile_pool(name="pp", bufs=8, space="PSUM") as pp:

            # ---- load weights/biases once (sync queue; small/early first) ----
            t_embA = wp.tile([128, 3 * 128], BF16)
            t_row4 = wp.tile([1, 128], BF16)
            t_iota = wp.tile([128, 3], F32)
            t_bc0 = wp.tile([128, 4], F32)
            t_bc1 = wp.tile([128, 4], F32)
            t_bhw = wp.tile([128, 4 * 8], F32)
            t_bpr = wp.tile([128, 4], F32)
            t_wc0 = wp.tile([128, 3 * WED], BF16)
            t_wc1 = wp.tile([128, 4 * 3 * WED], BF16)
            t_whw = wp.tile([128, 4 * 4 * 1024], BF16)
            t_wpr = wp.tile([128, 4 * WED], BF16)
            for t, d in ((t_embA, emb_lhs), (t_row4, emb_row4), (t_iota, iota_c),
                         (t_bc0, b_c0), (t_bc1, b_c1), (t_bhw, b_hw), (t_bpr, b_pr),
                         (t_wc0, w_c0), (t_wc1, w_c1), (t_whw, w_hw), (t_wpr, w_pr)):
                nc.sync.dma_start(out=t[:], in_=d[:])

            def conv_block(X, Y, wt, bt, n_ci):
                """Y[:, chunk m cols 1..T] = relu(conv(X) + b)."""
                for n in range(NCH):
                    for m in range(4):
                        ps = pp.tile([128, 512], F32, tag="ps", name="ps")
                        nmm = n_ci * 3
                        i = 0
                        for q in range(n_ci):
                            for k in range(3):
                                lhs = wt[:, (q * 3 + k) * WED + m * 128:(q * 3 + k) * WED + (m + 1) * 128]
                                nc.tensor.matmul(
                                    out=ps[:], lhsT=lhs,
                                    rhs=X[:, q * TP + n * 512 + k:q * TP + n * 512 + k + 512],
                                    start=(i == 0), stop=(i == nmm - 1))
                                i += 1
                        dst = Y[:, m * TP + 1 + n * 512:m * TP + 1 + (n + 1) * 512]
                        nc.scalar.activation(out=dst, in_=ps[:], func=RELU,
                                             bias=bt[:, m:m + 1], scale=1.0)

            def highway_layer(X, Y, bl, ytp=TP):
                """Y = g*relu(h) + (1-g)*X; X [128, 4*TP], Y [128, 4*ytp]."""
                for n in range(NCH):
                    pss = []
                    for m in range(8):
                        ps = pp.tile([128, 512], F32, tag="ps", name="ps")
                        for q in range(4):
                            base = (bl * 4 + q) * 1024 + m * 128
                            nc.tensor.matmul(
                                out=ps[:], lhsT=t_whw[:, base:base + 128],
                                rhs=X[:, q * TP + 1 + n * 512:q * TP + 1 + (n + 1) * 512],
                                start=(q == 0), stop=(q == 3))
                        pss.append(ps)
                    for c in range(4):
                        xs = X[:, c * TP + 1 + n * 512:c * TP + 1 + (n + 1) * 512]
                        h_t = tp.tile([128, 512], BF16, tag="h", name="h_t")
                        g_t = tp.tile([128, 512], BF16, tag="g", name="g_t")
                        d_t = tp.tile([128, 512], BF16, tag="d", name="d_t")
                        nc.scalar.activation(out=h_t[:], in_=pss[c][:], func=RELU,
                                             bias=t_bhw[:, bl * 8 + c:bl * 8 + c + 1], scale=1.0)
                        nc.scalar.activation(out=g_t[:], in_=pss[4 + c][:], func=SIGM,
                                             bias=t_bhw[:, bl * 8 + 4 + c:bl * 8 + 4 + c + 1], scale=1.0)
                        nc.vector.tensor_tensor(out=d_t[:], in0=h_t[:], in1=xs, op=SUB)
                        nc.vector.tensor_tensor(out=d_t[:], in0=d_t[:], in1=g_t[:], op=MUL)
                        ys = Y[:, c * ytp + 1 + n * 512:c * ytp + 1 + (n + 1) * 512]
                        nc.vector.tensor_tensor(out=ys, in0=d_t[:], in1=xs, op=ADD)

            scope = nc.named_scope

            for s in range(SEQ_PER_CORE):
                # ---------- embedding (one-hot matmul) ----------
                ctx = scope(f"s{s}_embed"); ctx.__enter__()
                t_tok = apool.tile([128, T], F16, tag="tok", name="t_tok")
                t_bpe = apool.tile([1, T], BF16, tag="bpe", name="t_bpe")
                t_am = apool.tile([128, 2 * T], BF16, tag="am", name="t_am")
                nc.scalar.dma_start(out=t_tok[:], in_=tok_bc[s])
                nc.scalar.dma_start(out=t_bpe[:], in_=bpe_row[s])
                nc.scalar.dma_start(out=t_am[:], in_=a_msk[s])

                x0 = apool.tile([128, TP], BF16, tag="x0", name="x0")
                nc.vector.memset(x0[:, 0:1], 0)
                nc.vector.memset(x0[:, TP - 1:TP], 0)
                for n in range(NCH):
                    oh1 = tp.tile([128, 512], BF16, tag="oh1", name="oh1")
                    oh2 = tp.tile([128, 512], BF16, tag="oh2", name="oh2")
                    oh3 = tp.tile([8, 512], BF16, tag="oh3", name="oh3")
                    tb = t_tok[:, n * 512:(n + 1) * 512]
                    nc.vector.tensor_scalar(out=oh1[:], in0=tb, scalar1=t_iota[:, 0:1],
                                            scalar2=None, op0=ISEQ)
                    nc.vector.tensor_scalar(out=oh2[:], in0=tb, scalar1=t_iota[:, 1:2],
                                            scalar2=None, op0=ISEQ)
                    nc.vector.tensor_scalar(out=oh3[:], in0=t_tok[0:8, n * 512:(n + 1) * 512],
                                            scalar1=t_iota[0:8, 2:3], scalar2=None, op0=ISEQ)
                    ps = pp.tile([128, 512], F32, tag="ps", name="ps")
                    nc.tensor.matmul(out=ps[:], lhsT=t_embA[:, 0:128], rhs=oh1[:],
                                     start=True, stop=False)
                    nc.tensor.matmul(out=ps[:], lhsT=t_embA[:, 128:256], rhs=oh2[:],
                                     start=False, stop=False)
                    nc.tensor.matmul(out=ps[:], lhsT=t_embA[0:8, 256:384], rhs=oh3[:],
                                     start=False, stop=False)
                    nc.tensor.matmul(out=ps[:], lhsT=t_row4[:], rhs=t_bpe[:, n * 512:(n + 1) * 512],
                                     start=False, stop=True)
                    nc.scalar.activation(out=x0[:, 1 + n * 512:1 + (n + 1) * 512],
                                         in_=ps[:], func=IDEN, bias=0.0, scale=1.0)
                ctx.__exit__(None, None, None)

                def act_buf(tag, w=TP, extra_halo=0):
                    b = apool.tile([128, 4 * w], BF16, tag=tag, name=tag)
                    for q in range(4):
                        nc.vector.memset(b[:, q * w:q * w + 1], 0)
                        nc.vector.memset(b[:, q * w + 1 + T:(q + 1) * w], 0)
                    return b

                # ---------- conv0 + highway block 0 ----------
                with scope(f"s{s}_conv0"):
                    x1 = act_buf("actA")
                    conv_block(x0, x1, t_wc0, t_bc0, 1)
                with scope(f"s{s}_hw0l0"):
                    x1b = act_buf("actB")
                    highway_layer(x1, x1b, 0)
                with scope(f"s{s}_hw0l1"):
                    x1c = act_buf("actC")
                    highway_layer(x1b, x1c, 1)

                # ---------- conv1 (+res) + highway block 1 ----------
                ctx = scope(f"s{s}_conv1"); ctx.__enter__()
                x2p = act_buf("actA")
                for n in range(NCH):
                    for m in range(4):
                        ps = pp.tile([128, 512], F32, tag="ps", name="ps")
                        i = 0
                        for q in range(4):
                            for k in range(3):
                                lhs = t_wc1[:, (q * 3 + k) * WED + m * 128:(q * 3 + k) * WED + (m + 1) * 128]
                                nc.tensor.matmul(
                                    out=ps[:], lhsT=lhs,
                                    rhs=x1c[:, q * TP + n * 512 + k:q * TP + n * 512 + k + 512],
                                    start=(i == 0), stop=(i == 11))
                                i += 1
                        r_t = tp.tile([128, 512], BF16, tag="h", name="r_t")
                        nc.scalar.activation(out=r_t[:], in_=ps[:], func=RELU,
                                             bias=t_bc1[:, m:m + 1], scale=1.0)
                        xs = x1c[:, m * TP + 1 + n * 512:m * TP + 1 + (n + 1) * 512]
                        nc.vector.tensor_tensor(
                            out=x2p[:, m * TP + 1 + n * 512:m * TP + 1 + (n + 1) * 512],
                            in0=r_t[:], in1=xs, op=ADD)
                ctx.__exit__(None, None, None)

                with scope(f"s{s}_hw1l0"):
                    x2b = act_buf("actB")
                    highway_layer(x2p, x2b, 2)
                with scope(f"s{s}_hw1l1"):
                    x2 = act_buf("actC", w=TP2)
                    highway_layer(x2b, x2, 3, ytp=TP2)

                # ---------- ragged max pool + projection, pipelined per T-chunk ----------
                ctx = scope(f"s{s}_poolproj"); ctx.__enter__()
                msel = apool.tile([128, 4 * T], BF16, tag="actB", name="msel")
                for n in range(NCH):
                    lo, hi = n * 512, (n + 1) * 512
                    for c in range(4):
                        base = c * TP2 + 1
                        s1 = tp.tile([128, 512], BF16, tag="s1", name="s1")
                        s2 = tp.tile([128, 512], BF16, tag="s2", name="s2")
                        nc.vector.tensor_tensor(out=s1[:], in0=x2[:, base + 1 + lo:base + 1 + hi],
                                                in1=t_am[:, lo:hi], op=ADD)
                        nc.vector.tensor_tensor(out=s2[:], in0=x2[:, base + 2 + lo:base + 2 + hi],
                                                in1=t_am[:, T + lo:T + hi], op=ADD)
                        nc.vector.tensor_tensor(out=s1[:], in0=s1[:], in1=s2[:], op=MAX)
                        nc.vector.tensor_tensor(out=msel[:, c * T + lo:c * T + hi],
                                                in0=s1[:], in1=x2[:, base + lo:base + hi], op=MAX)
                    for m in range(4):
                        ps = pp.tile([128, 512], F32, tag="ps", name="ps")
                        for q in range(4):
                            nc.tensor.matmul(
                                out=ps[:], lhsT=t_wpr[:, q * WED + m * 128:q * WED + (m + 1) * 128],
                                rhs=msel[:, q * T + lo:q * T + hi],
                                start=(q == 0), stop=(q == 3))
                        o_t = tp.tile([128, 512], F32, tag="o", name="o_t", bufs=4)
                        nc.scalar.activation(out=o_t[:], in_=ps[:], func=IDEN,
                                             bias=t_bpr[:, m:m + 1], scale=1.0)
                        nc.sync.dma_start(out=out[s, m * 128:(m + 1) * 128, lo:hi], in_=o_t[:])
                ctx.__exit__(None, None, None)

    nc.compile()
    return nc


def _prep_inputs(inputs):
    """Host-side: shard + convert to the kernel's DRAM tensor layouts."""
    byte_tokens = np.asarray(inputs["byte_tokens"], np.int64)
    bpe_mask = np.asarray(inputs["bpe_mask"], bool)
    pool_lengths = np.asarray(inputs["pool_lengths"], np.int64)
    tok_emb = np.asarray(inputs["tok_emb"], np.float32)

    def bf(x):
        return np.ascontiguousarray(np.asarray(x, np.float32).astype(_BF16_NP))

    conv0_W = np.asarray(inputs["conv0_W"], np.float32)   # [3,128,512]
    conv1_W = np.asarray(inputs["conv1_W"], np.float32)   # [3,512,512]
    hw0_W = np.asarray(inputs["hw0_W"], np.float32)       # [2,1024,512]
    hw1_W = np.asarray(inputs["hw1_W"], np.float32)
    proj_W = np.asarray(inputs["proj_W"], np.float32)     # [512,512]

    w_c0 = bf(conv0_W.transpose(1, 0, 2).reshape(128, 3 * WED))
    w_c1 = bf(conv1_W.transpose(1, 0, 2).reshape(4, 128, 3, WED)
              .transpose(1, 0, 2, 3).reshape(128, 4 * 3 * WED))
    whw = np.empty((128, 16, 1024), np.float32)
    for bl, (blk, lay) in enumerate(((hw0_W, 0), (hw0_W, 1), (hw1_W, 0), (hw1_W, 1))):
        wt = blk[lay].T  # [512, 1024]
        for q in range(4):
            whw[:, bl * 4 + q, :] = wt[q * 128:(q + 1) * 128]
    w_hw = bf(whw.reshape(128, 16 * 1024))
    w_pr = bf(proj_W.T.reshape(4, 128, WED).transpose(1, 0, 2).reshape(128, 4 * WED))

    def colchunks(b):  # [512] -> [128, 4]
        return np.ascontiguousarray(np.asarray(b, np.float32).reshape(4, 128).T)

    b_c0 = colchunks(inputs["conv0_b"])
    b_c1 = colchunks(inputs["conv1_b"])
    bhw = np.empty((128, 4, 8), np.float32)
    for bl, (blk, lay) in enumerate((("hw0_b", 0), ("hw0_b", 1), ("hw1_b", 0), ("hw1_b", 1))):
        b = np.asarray(inputs[blk], np.float32)[lay]      # [1024]
        bhw[:, bl, 0:4] = b[:512].reshape(4, 128).T
        bhw[:, bl, 4:8] = b[512:1024].reshape(4, 128).T
    b_hw = np.ascontiguousarray(bhw.reshape(128, 32))
    b_pr = colchunks(inputs["proj_b"])

    # embedding table as lhsT row-chunks [128, 3*128]
    emb_lhs = np.zeros((128, 3 * 128), np.float32)
    emb_lhs[:, 0:128] = tok_emb[0:128]
    emb_lhs[:, 128:256] = tok_emb[128:256]
    emb_lhs[0:8, 256:384] = tok_emb[256:264]
    emb_lhs = bf(emb_lhs)
    emb_row4 = bf(tok_emb[BPE_MASK_IDX:BPE_MASK_IDX + 1, :])  # [1, 128]
    iota_c = np.empty((128, 3), np.float32)
    p = np.arange(128)
    for j in range(3):
        iota_c[:, j] = (j * 128 + p).astype(np.float32)

    shared = dict(emb_lhs=emb_lhs, emb_row4=emb_row4, iota_c=iota_c,
                  w_c0=w_c0, w_c1=w_c1, w_hw=w_hw, w_pr=w_pr,
                  b_c0=b_c0, b_c1=b_c1, b_hw=b_hw, b_pr=b_pr)

    in_maps = []
    meta = []
    for core in range(N_CORES):
        m = dict(shared)
        tok = np.empty((SEQ_PER_CORE, 128, T), _F16_NP)
        bpe = np.empty((SEQ_PER_CORE, 1, T), _BF16_NP)
        amsk = np.empty((SEQ_PER_CORE, 128, 2 * T), _BF16_NP)
        for s in range(SEQ_PER_CORE):
            b = core * SEQ_PER_CORE + s
            tok[s] = np.broadcast_to(byte_tokens[b].astype(_F16_NP), (128, T))
            bpe[s, 0] = (bpe_mask[b]).astype(_BF16_NP)
            pl = pool_lengths[b]
            cum = np.cumsum(pl)
            s_w = (cum - pl)
            a1 = np.full(T, NEG_BIG, np.float32)
            a2 = np.full(T, NEG_BIG, np.float32)
            st = s_w[pl > 1]
            a1[st[st < T]] = 0.0
            st = s_w[pl > 2]
            a2[st[st < T]] = 0.0
            amsk[s, :, 0:T] = np.broadcast_to(a1.astype(_BF16_NP), (128, T))
            amsk[s, :, T:2 * T] = np.broadcast_to(a2.astype(_BF16_NP), (128, T))
            meta.append((s_w, pl))
        m["tok_bc"] = tok
        m["bpe_row"] = bpe
        m["a_msk"] = amsk
        in_maps.append(m)
    return in_maps, meta


def kernel(**inputs) -> np.ndarray:
    from concourse.bass_utils import run_bass_kernel_spmd

    if "nc" not in _CACHE:
        _CACHE["nc"] = _build_program()
    nc = _CACHE["nc"]

    in_maps, meta = _prep_inputs(inputs)
    res = run_bass_kernel_spmd(nc, in_maps, list(range(N_CORES)))

    proj_b = np.asarray(inputs["proj_b"], np.float32)
    full = np.empty((BSZ, NW, WED), np.float32)
    for core in range(N_CORES):
        o = np.asarray(res.results[core]["out"], np.float32)  # [2, 512, T]
        for s in range(SEQ_PER_CORE):
            b = core * SEQ_PER_CORE + s
            s_w, pl = meta[b]
            cols = np.clip(s_w, 0, T - 1)
            full[b] = o[s][:, cols].T
            if (pl == 0).any():
                full[b][pl == 0] = proj_b
    return full
